# revision 17
# baseline (speedup 1.0000x reference)
"""Multi-head attention Trainium2 kernel (8 NeuronCores).

Sharding: 8 cores = 4 batches x 2 sequence-halves (data parallel, no
collectives).  Each core computes, for its (batch, s-half):
  - q/k/v projections for all 16 heads in transposed layout
  - scoresT = k_h^T-chunks (x) q_h  per head, softmax without max-subtraction
    (scores are bounded ~ +-3.5 for this problem's input distribution)
  - ctxT via col-packed matmuls with v as the stationary operand; the softmax
    denominator comes from a ones-stationary matmul replicated across
    partitions so the division is partition-aligned on VectorE
  - row-wise output projection out = concat(ctx) @ Wo
Host side: transpose/cast/shard inputs, gather per-core outputs.

Bias handling: bq/bk/bv/bo are all zero for this problem.  bo and bv have
exact host-side corrections (softmax rows sum to 1 so a v-bias shifts ctx by
exactly bv); bq/bk would require a device change and are asserted zero.
"""

import os

import numpy as np
import ml_dtypes

B, S, E, H, DH = 4, 2048, 1024, 16, 64
SL = S // 2          # per-core local sequence (s-half)
NE = E // 128        # e-tiles (contraction)
NT = S // 128        # t-tiles (keys)
NP = H // 2          # head-pairs; pair j = heads (2j, 2j+1) = hd rows 128j..128j+128
NCORES = 8

_cache = {}


def _build(reps=1):
    import concourse.mybir as mybir
    import concourse.tile as tile
    from concourse import bacc
    from contextlib import ExitStack

    f32 = mybir.dt.float32
    f32r = mybir.dt.float32r
    bf16 = mybir.dt.bfloat16
    EXP = mybir.ActivationFunctionType.Exp

    nc = bacc.Bacc("TRN2", target_bir_lowering=False, debug=False,
                   num_devices=NCORES)

    # xT column order per core: own s-half first (t-permutation is harmless
    # for attention sums), so the q-projection always reads columns 0:SL.
    xT_d = nc.dram_tensor("xT", [E, S], bf16, kind="ExternalInput")
    wq_d = nc.dram_tensor("wq", [E, E], bf16, kind="ExternalInput")
    wk_d = nc.dram_tensor("wk", [E, E], bf16, kind="ExternalInput")
    wv_d = nc.dram_tensor("wv", [E, E], bf16, kind="ExternalInput")
    wo_d = nc.dram_tensor("wo", [E, E], bf16, kind="ExternalInput")
    out_d = nc.dram_tensor("out", [SL, E], f32, kind="ExternalOutput")

    with tile.TileContext(nc) as tc, ExitStack() as top:
        singles = top.enter_context(tc.tile_pool(name="singles", bufs=1))
        sb_out = top.enter_context(tc.tile_pool(name="sb_out", bufs=2))
        ps_s = top.enter_context(tc.tile_pool(name="ps_s", bufs=2, space="PSUM"))
        ps_qk = top.enter_context(tc.tile_pool(name="ps_qk", bufs=2, space="PSUM"))
        ps_cd = top.enter_context(tc.tile_pool(name="ps_cd", bufs=2, space="PSUM"))

        xT_sb = singles.tile([128, NE, S], bf16)
        qT_sb = singles.tile([128, NP, SL], bf16)
        ctx_sb = singles.tile([128, NP, SL], bf16)
        wo_sb = singles.tile([128, NP, E], bf16)
        xT_r = xT_d.rearrange("(eo ei) t -> ei eo t", ei=128)
        for e in range(NE):
            nc.sync.dma_start(out=xT_sb[:, e, :], in_=xT_r[:, e, :])
        nc.gpsimd.dma_start(
            out=wo_sb[:], in_=wo_d.rearrange("(ho hi) e -> hi ho e", hi=128))

        for _rep in range(reps):
          with ExitStack() as attn:
            sb_w = attn.enter_context(tc.tile_pool(name="sb_w", bufs=2))
            sb_kt = attn.enter_context(tc.tile_pool(name="sb_kt", bufs=2))
            sb_v = attn.enter_context(tc.tile_pool(name="sb_v", bufs=2))
            sb_pt = attn.enter_context(tc.tile_pool(name="sb_pt", bufs=3))
            sb_nm = attn.enter_context(tc.tile_pool(name="sb_nm", bufs=2))

            v_tiles = {}
            kT_tiles = {}

            def project_v(o):
                # project v for octet o (8 heads = pairs 4o..4o+3).
                # v_sb slot hh holds [v_h | ones] for even heads and
                # [ones | v_h] for odd heads: the ctx matmul then yields
                # ctx and the softmax denominator in one pass.
                wv_sb = sb_w.tile([128, NE, 512], bf16, tag="wv")
                wv_r = wv_d[:, o * 512:(o + 1) * 512].rearrange(
                    "(eo ei) h -> ei eo h", ei=128)
                for e in range(NE):
                    nc.gpsimd.dma_start(out=wv_sb[:, e, :], in_=wv_r[:, e, :])
                v_sb = sb_v.tile([128, NT, 8, 128], bf16, tag="v")
                nc.vector.memset(v_sb[:, :, 0:8:2, 64:128], 1.0)
                nc.vector.memset(v_sb[:, :, 1:8:2, 0:64], 1.0)
                for t in range(NT):
                    ps_v = ps_s.tile([128, 512], f32, tag="ps_s")
                    for e in range(NE):
                        nc.tensor.matmul(
                            ps_v[:], xT_sb[:, e, t * 128:(t + 1) * 128],
                            wv_sb[:, e, :],
                            start=(e == 0), stop=(e == NE - 1))
                    pv = ps_v.rearrange("p (h d) -> p h d", d=64)
                    nc.vector.tensor_copy(v_sb[:, t, 0:8:2, 0:64], pv[:, 0:8:2, :])
                    nc.vector.tensor_copy(v_sb[:, t, 1:8:2, 64:128], pv[:, 1:8:2, :])
                v_tiles[o] = v_sb

            def project_qk(j):
                if j % 4 == 0:
                    project_v(j // 4)
                wq_sb = sb_w.tile([128, NE, 128], bf16, tag="wq")
                wk_sb = sb_w.tile([128, NE, 128], bf16, tag="wk")
                wq_r = wq_d[:, j * 128:(j + 1) * 128].rearrange(
                    "(eo ei) h -> ei eo h", ei=128)
                wk_r = wk_d[:, j * 128:(j + 1) * 128].rearrange(
                    "(eo ei) h -> ei eo h", ei=128)
                for e in range(NE):
                    nc.gpsimd.dma_start(out=wq_sb[:, e, :], in_=wq_r[:, e, :])
                    nc.gpsimd.dma_start(out=wk_sb[:, e, :], in_=wk_r[:, e, :])

                # qT[j] : [128 hd, SL]
                for sc in range(2):
                    ps_q = ps_qk.tile([128, 512], f32, tag="ps_qk")
                    for e in range(NE):
                        nc.tensor.matmul(
                            ps_q[:], wq_sb[:, e, :],
                            xT_sb[:, e, sc * 512:(sc + 1) * 512],
                            start=(e == 0), stop=(e == NE - 1))
                    nc.vector.tensor_copy(qT_sb[:, j, sc * 512:(sc + 1) * 512], ps_q[:])

                # kT[j] : [128 hd, S]
                kT_sb = sb_kt.tile([128, S], bf16, tag="kt")
                for tch in range(4):
                    ps_k = ps_qk.tile([128, 512], f32, tag="ps_qk")
                    for e in range(NE):
                        nc.tensor.matmul(
                            ps_k[:], wk_sb[:, e, :],
                            xT_sb[:, e, tch * 512:(tch + 1) * 512],
                            start=(e == 0), stop=(e == NE - 1))
                    nc.vector.tensor_copy(kT_sb[:, tch * 512:(tch + 1) * 512], ps_k[:])
                kT_tiles[j] = kT_sb

            project_qk(0)
            for j in range(NP):
                if j + 1 < NP:
                    project_qk(j + 1)
                jj = j % 4
                v_sb = v_tiles[j // 4]
                kT_sb = kT_tiles.pop(j)

                # attention for pair j, per s-chunk of 512
                for sc in range(2):
                    # ps_ca: head A = [v|ones] -> ctx rows 0:64, denom rows 64:128
                    # ps_cb: head B = [ones|v] -> denom rows 0:64, ctx rows 64:128
                    ps_ca = ps_cd.tile([128, 512], f32, tag="ps_cd")
                    ps_cb = ps_cd.tile([128, 512], f32, tag="ps_cd")
                    for t in range(NT):
                        ps_sc = ps_s.tile([128, 1024], f32, tag="ps_s")
                        # scoresT for heads A/B, row-packed (K=64 each)
                        nc.tensor.matmul(
                            ps_sc[:, 0:512],
                            kT_sb[0:64, t * 128:(t + 1) * 128],
                            qT_sb[0:64, j, sc * 512:(sc + 1) * 512],
                            start=True, stop=True, tile_position=(0, 0))
                        nc.tensor.matmul(
                            ps_sc[:, 512:1024],
                            kT_sb[64:128, t * 128:(t + 1) * 128],
                            qT_sb[64:128, j, sc * 512:(sc + 1) * 512],
                            start=True, stop=True, tile_position=(64, 0))
                        pt_t = sb_pt.tile([128, 1024], bf16, tag="pt")
                        nc.scalar.activation(pt_t[:], ps_sc[:], EXP, scale=0.125)
                        stt, stp = (t == 0), (t == NT - 1)
                        nc.tensor.matmul(
                            ps_ca[:], v_sb[:, t, jj * 2, :], pt_t[:, 0:512],
                            start=stt, stop=stp)
                        nc.tensor.matmul(
                            ps_cb[:], v_sb[:, t, jj * 2 + 1, :], pt_t[:, 512:1024],
                            start=stt, stop=stp)
                    # normalize head A: denom replicated at rows 64:128; move one
                    # row to partition 0, reciprocal, broadcast to rows 0:64
                    tA = sb_nm.tile([128, 512], f32, tag="tA")
                    rA = sb_nm.tile([1, 512], f32, tag="rA")
                    rbA = sb_nm.tile([64, 512], f32, tag="rbA")
                    nc.vector.tensor_copy(tA[64:65, :], ps_ca[64:65, :])
                    nc.sync.dma_start(out=tA[0:1, :], in_=tA[64:65, :])
                    nc.vector.reciprocal_approx_fast(rA[0:1, :], tA[0:1, :])
                    nc.gpsimd.partition_broadcast(rbA[:, :], rA[0:1, :])
                    nc.vector.tensor_mul(
                        ctx_sb[0:64, j, sc * 512:(sc + 1) * 512],
                        ps_ca[0:64, :], rbA[:, :])
                    # normalize head B: denom at row 0 already
                    rB = sb_nm.tile([1, 512], f32, tag="rB")
                    rbB = sb_nm.tile([128, 512], f32, tag="rbB")
                    nc.vector.reciprocal_approx_fast(rB[0:1, :], ps_cb[0:1, :])
                    nc.gpsimd.partition_broadcast(rbB[:, :], rB[0:1, :])
                    nc.vector.tensor_mul(
                        ctx_sb[64:128, j, sc * 512:(sc + 1) * 512],
                        ps_cb[64:128, :], rbB[64:128, :])

          # output projection: out[s, e] = sum_j ctxT_j^T @ wo_j
          if True:
            for stile in range(8):
                for sc in range(2):
                    ps_o = ps_qk.tile([128, 512], f32, tag="ps_qk")
                    for j in range(NP):
                        nc.tensor.matmul(
                            ps_o[:], ctx_sb[:, j, stile * 128:(stile + 1) * 128],
                            wo_sb[:, j, sc * 512:(sc + 1) * 512],
                            start=(j == 0), stop=(j == NP - 1))
                    ot = sb_out.tile([128, 512], f32, tag="out")
                    nc.vector.tensor_copy(ot[:], ps_o[:])
                    nc.sync.dma_start(
                        out=out_d[stile * 128:(stile + 1) * 128,
                                  sc * 512:(sc + 1) * 512],
                        in_=ot[:])

    nc.compile()
    return nc


def _prep(xs, Wq, Wk, Wv, Wo):
    bf = ml_dtypes.bfloat16
    wq16 = np.ascontiguousarray(Wq.transpose(1, 0, 2).reshape(E, E)).astype(bf)
    wk16 = np.ascontiguousarray(Wk.transpose(1, 0, 2).reshape(E, E)).astype(bf)
    wv16 = np.ascontiguousarray(Wv.transpose(1, 0, 2).reshape(E, E)).astype(bf)
    wo32 = np.ascontiguousarray(Wo).astype(bf)
    in_maps = []
    xT_b = [np.ascontiguousarray(xs[b].T).astype(bf) for b in range(B)]
    for c in range(NCORES):
        b, half = divmod(c, 2)
        xT = xT_b[b] if half == 0 else np.ascontiguousarray(
            np.concatenate([xT_b[b][:, SL:], xT_b[b][:, :SL]], axis=1))
        in_maps.append({"xT": xT, "wq": wq16, "wk": wk16, "wv": wv16, "wo": wo32})
    return in_maps


def kernel(xs, Wq, bq, Wk, bk, Wv, bv, Wo, bo):
    from concourse.bass_utils import run_bass_kernel_spmd

    if "nc" not in _cache:
        _cache["nc"] = _build()
    nc = _cache["nc"]

    xs = np.asarray(xs, dtype=np.float32)
    Wq = np.asarray(Wq, dtype=np.float32)
    Wk = np.asarray(Wk, dtype=np.float32)
    Wv = np.asarray(Wv, dtype=np.float32)
    Wo = np.asarray(Wo, dtype=np.float32)
    bq = np.asarray(bq, dtype=np.float32)
    bk = np.asarray(bk, dtype=np.float32)
    bv = np.asarray(bv, dtype=np.float32)
    bo = np.asarray(bo, dtype=np.float32)
    assert not (np.any(bq) or np.any(bk)), "nonzero bq/bk not supported"

    in_maps = _prep(xs, Wq, Wk, Wv, Wo)

    trace = bool(int(os.environ.get("BASS_KERNEL_TRACE", "0")))
    kw = {}
    if trace:
        kw = dict(trace=True, trace_cores=[0])
    res = run_bass_kernel_spmd(nc, in_maps, core_ids=list(range(NCORES)), **kw)
    if trace and res.exec_time_ns is not None:
        print(f"HW exec time: {res.exec_time_ns} ns")
        if res.instructions_and_trace is not None:
            print("trace:", res.instructions_and_trace[1])

    out = np.empty((B, S, E), dtype=np.float32)
    for c in range(NCORES):
        b, half = divmod(c, 2)
        out[b, half * SL:(half + 1) * SL, :] = res.results[c]["out"]

    # exact host-side correction for v/output biases (zero in this problem)
    if np.any(bv) or np.any(bo):
        out += bv.reshape(E) @ Wo + bo
    return out


# revision 20
# speedup vs baseline: 1.1045x; 1.1045x over previous
"""Multi-head attention Trainium2 kernel (8 NeuronCores).

Sharding: 8 cores = 4 batches x 2 sequence-halves (data parallel, no
collectives).  Each core computes, for its (batch, s-half):
  - q/k/v projections for all 16 heads in transposed layout
  - scoresT = k_h^T-chunks (x) q_h  per head, softmax without max-subtraction
    (scores are bounded ~ +-3.5 for this problem's input distribution)
  - ctxT via col-packed matmuls with v as the stationary operand; the softmax
    denominator comes from a ones-stationary matmul replicated across
    partitions so the division is partition-aligned on VectorE
  - row-wise output projection out = concat(ctx) @ Wo
Host side: transpose/cast/shard inputs, gather per-core outputs.

Bias handling: bq/bk/bv/bo are all zero for this problem.  bo and bv have
exact host-side corrections (softmax rows sum to 1 so a v-bias shifts ctx by
exactly bv); bq/bk would require a device change and are asserted zero.
"""

import os

import numpy as np
import ml_dtypes

B, S, E, H, DH = 4, 2048, 1024, 16, 64
SL = S // 2          # per-core local sequence (s-half)
NE = E // 128        # e-tiles (contraction)
NT = S // 128        # t-tiles (keys)
NP = H // 2          # head-pairs; pair j = heads (2j, 2j+1) = hd rows 128j..128j+128
NCORES = 8

_cache = {}


def _build(reps=1):
    import concourse.mybir as mybir
    import concourse.tile as tile
    from concourse import bacc
    from contextlib import ExitStack

    f32 = mybir.dt.float32
    f32r = mybir.dt.float32r
    bf16 = mybir.dt.bfloat16
    EXP = mybir.ActivationFunctionType.Exp

    nc = bacc.Bacc("TRN2", target_bir_lowering=False, debug=False,
                   num_devices=NCORES)

    # xT column order per core: own s-half first (t-permutation is harmless
    # for attention sums), so the q-projection always reads columns 0:SL.
    xT_d = nc.dram_tensor("xT", [E, S], bf16, kind="ExternalInput")
    wq_d = nc.dram_tensor("wq", [E, E], bf16, kind="ExternalInput")
    wk_d = nc.dram_tensor("wk", [E, E], bf16, kind="ExternalInput")
    wv_d = nc.dram_tensor("wv", [E, E], bf16, kind="ExternalInput")
    wo_d = nc.dram_tensor("wo", [E, E], bf16, kind="ExternalInput")
    out_d = nc.dram_tensor("out", [SL, E], f32, kind="ExternalOutput")

    with tile.TileContext(nc) as tc, ExitStack() as top:
        singles = top.enter_context(tc.tile_pool(name="singles", bufs=1))
        sb_out = top.enter_context(tc.tile_pool(name="sb_out", bufs=2))
        ps_s = top.enter_context(tc.tile_pool(name="ps_s", bufs=2, space="PSUM"))
        ps_qk = top.enter_context(tc.tile_pool(name="ps_qk", bufs=2, space="PSUM"))
        ps_cd = top.enter_context(tc.tile_pool(name="ps_cd", bufs=2, space="PSUM"))

        xT_sb = singles.tile([128, NE, S], bf16)
        qT_sb = singles.tile([128, NP, SL], bf16)
        ctx_sb = singles.tile([128, NP, SL], bf16)
        wo_sb = singles.tile([128, NP, E], bf16)
        xT_r = xT_d.rearrange("(eo ei) t -> ei eo t", ei=128)
        for e in range(NE):
            eng = nc.sync if e % 2 == 0 else nc.gpsimd
            eng.dma_start(out=xT_sb[:, e, :], in_=xT_r[:, e, :])
        nc.gpsimd.dma_start(
            out=wo_sb[:], in_=wo_d.rearrange("(ho hi) e -> hi ho e", hi=128))

        for _rep in range(reps):
          with ExitStack() as attn:
            sb_w = attn.enter_context(tc.tile_pool(name="sb_w", bufs=2))
            sb_kt = attn.enter_context(tc.tile_pool(name="sb_kt", bufs=2))
            sb_v = attn.enter_context(tc.tile_pool(name="sb_v", bufs=2))
            sb_pt = attn.enter_context(tc.tile_pool(name="sb_pt", bufs=4))
            sb_nm = attn.enter_context(tc.tile_pool(name="sb_nm", bufs=1))

            v_tiles = {}
            kT_tiles = {}

            def project_v(o):
                # project v for octet o (8 heads = pairs 4o..4o+3).
                # v_sb slot hh holds [v_h | ones] for even heads and
                # [ones | v_h] for odd heads: the ctx matmul then yields
                # ctx and the softmax denominator in one pass.
                wv_sb = sb_w.tile([128, NE, 512], bf16, tag="wv")
                wv_r = wv_d[:, o * 512:(o + 1) * 512].rearrange(
                    "(eo ei) h -> ei eo h", ei=128)
                for e in range(NE):
                    nc.gpsimd.dma_start(out=wv_sb[:, e, :], in_=wv_r[:, e, :])
                v_sb = sb_v.tile([128, NT, 8, 128], bf16, tag="v")
                nc.vector.memset(v_sb[:, :, 0:8:2, 64:128], 1.0)
                nc.vector.memset(v_sb[:, :, 1:8:2, 0:64], 1.0)
                for t in range(NT):
                    ps_v = ps_s.tile([128, 512], f32, tag="ps_s")
                    for e in range(NE):
                        nc.tensor.matmul(
                            ps_v[:], xT_sb[:, e, t * 128:(t + 1) * 128],
                            wv_sb[:, e, :],
                            start=(e == 0), stop=(e == NE - 1))
                    pv = ps_v.rearrange("p (h d) -> p h d", d=64)
                    nc.vector.tensor_copy(v_sb[:, t, 0:8:2, 0:64], pv[:, 0:8:2, :])
                    nc.vector.tensor_copy(v_sb[:, t, 1:8:2, 64:128], pv[:, 1:8:2, :])
                v_tiles[o] = v_sb

            def project_qk(j):
                if j % 4 == 0:
                    project_v(j // 4)
                wq_sb = sb_w.tile([128, NE, 128], bf16, tag="wq")
                wk_sb = sb_w.tile([128, NE, 128], bf16, tag="wk")
                wq_r = wq_d[:, j * 128:(j + 1) * 128].rearrange(
                    "(eo ei) h -> ei eo h", ei=128)
                wk_r = wk_d[:, j * 128:(j + 1) * 128].rearrange(
                    "(eo ei) h -> ei eo h", ei=128)
                for e in range(NE):
                    nc.gpsimd.dma_start(out=wq_sb[:, e, :], in_=wq_r[:, e, :])
                    nc.gpsimd.dma_start(out=wk_sb[:, e, :], in_=wk_r[:, e, :])

                # qT[j] : [128 hd, SL]
                for sc in range(2):
                    ps_q = ps_qk.tile([128, 512], f32, tag="ps_qk")
                    for e in range(NE):
                        nc.tensor.matmul(
                            ps_q[:], wq_sb[:, e, :],
                            xT_sb[:, e, sc * 512:(sc + 1) * 512],
                            start=(e == 0), stop=(e == NE - 1))
                    nc.vector.tensor_copy(qT_sb[:, j, sc * 512:(sc + 1) * 512], ps_q[:])

                # kT[j] : [128 hd, S]
                kT_sb = sb_kt.tile([128, S], bf16, tag="kt")
                for tch in range(4):
                    ps_k = ps_qk.tile([128, 512], f32, tag="ps_qk")
                    for e in range(NE):
                        nc.tensor.matmul(
                            ps_k[:], wk_sb[:, e, :],
                            xT_sb[:, e, tch * 512:(tch + 1) * 512],
                            start=(e == 0), stop=(e == NE - 1))
                    nc.vector.tensor_copy(kT_sb[:, tch * 512:(tch + 1) * 512], ps_k[:])
                kT_tiles[j] = kT_sb

            project_qk(0)
            for j in range(NP):
                if j + 1 < NP:
                    project_qk(j + 1)
                jj = j % 4
                v_sb = v_tiles[j // 4]
                kT_sb = kT_tiles.pop(j)

                # attention for pair j, per s-chunk of 512
                for sc in range(2):
                    # ps_ca: head A = [v|ones] -> ctx rows 0:64, denom rows 64:128
                    # ps_cb: head B = [ones|v] -> denom rows 0:64, ctx rows 64:128
                    ps_ca = ps_cd.tile([128, 512], f32, tag="ps_cd")
                    ps_cb = ps_cd.tile([128, 512], f32, tag="ps_cd")
                    for t in range(NT):
                        ps_sc = ps_s.tile([128, 1024], f32, tag="ps_s")
                        # scoresT for heads A/B, row-packed (K=64 each)
                        nc.tensor.matmul(
                            ps_sc[:, 0:512],
                            kT_sb[0:64, t * 128:(t + 1) * 128],
                            qT_sb[0:64, j, sc * 512:(sc + 1) * 512],
                            start=True, stop=True, tile_position=(0, 0))
                        nc.tensor.matmul(
                            ps_sc[:, 512:1024],
                            kT_sb[64:128, t * 128:(t + 1) * 128],
                            qT_sb[64:128, j, sc * 512:(sc + 1) * 512],
                            start=True, stop=True, tile_position=(64, 0))
                        pt_t = sb_pt.tile([128, 1024], bf16, tag="pt")
                        nc.scalar.activation(pt_t[:], ps_sc[:], EXP, scale=0.125)
                        stt, stp = (t == 0), (t == NT - 1)
                        nc.tensor.matmul(
                            ps_ca[:], v_sb[:, t, jj * 2, :], pt_t[:, 0:512],
                            start=stt, stop=stp)
                        nc.tensor.matmul(
                            ps_cb[:], v_sb[:, t, jj * 2 + 1, :], pt_t[:, 512:1024],
                            start=stt, stop=stp)
                    # evacuate both psums in one copy each (releases the
                    # ps_cd slots for the next s-chunk immediately), then
                    # normalize off the SBUF copies
                    tA = sb_nm.tile([128, 512], f32, tag="tA")
                    tB = sb_nm.tile([128, 512], f32, tag="tB")
                    nc.vector.tensor_copy(tA[:, :], ps_ca[:, :])
                    nc.vector.tensor_copy(tB[:, :], ps_cb[:, :])
                    # head A: denom replicated at rows 64:128; move one row to
                    # partition 0, reciprocal, broadcast to rows 0:64
                    rA = sb_nm.tile([1, 512], f32, tag="rA")
                    rbA = sb_nm.tile([64, 512], f32, tag="rbA")
                    nc.sync.dma_start(out=rA[0:1, :], in_=tA[64:65, :])
                    nc.vector.reciprocal_approx_fast(rA[0:1, :], rA[0:1, :])
                    nc.gpsimd.partition_broadcast(rbA[:, :], rA[0:1, :])
                    nc.vector.tensor_mul(
                        ctx_sb[0:64, j, sc * 512:(sc + 1) * 512],
                        tA[0:64, :], rbA[:, :])
                    # head B: denom at row 0 already
                    rB = sb_nm.tile([1, 512], f32, tag="rB")
                    rbB = sb_nm.tile([128, 512], f32, tag="rbB")
                    nc.vector.reciprocal_approx_fast(rB[0:1, :], tB[0:1, :])
                    nc.gpsimd.partition_broadcast(rbB[:, :], rB[0:1, :])
                    nc.vector.tensor_mul(
                        ctx_sb[64:128, j, sc * 512:(sc + 1) * 512],
                        tB[64:128, :], rbB[64:128, :])

          # output projection: out[s, e] = sum_j ctxT_j^T @ wo_j
          if True:
            for stile in range(8):
                for sc in range(2):
                    ps_o = ps_qk.tile([128, 512], f32, tag="ps_qk")
                    for j in range(NP):
                        nc.tensor.matmul(
                            ps_o[:], ctx_sb[:, j, stile * 128:(stile + 1) * 128],
                            wo_sb[:, j, sc * 512:(sc + 1) * 512],
                            start=(j == 0), stop=(j == NP - 1))
                    ot = sb_out.tile([128, 512], f32, tag="out")
                    nc.vector.tensor_copy(ot[:], ps_o[:])
                    nc.sync.dma_start(
                        out=out_d[stile * 128:(stile + 1) * 128,
                                  sc * 512:(sc + 1) * 512],
                        in_=ot[:])

    nc.compile()
    return nc


def _prep(xs, Wq, Wk, Wv, Wo):
    bf = ml_dtypes.bfloat16
    wq16 = np.ascontiguousarray(Wq.transpose(1, 0, 2).reshape(E, E)).astype(bf)
    wk16 = np.ascontiguousarray(Wk.transpose(1, 0, 2).reshape(E, E)).astype(bf)
    wv16 = np.ascontiguousarray(Wv.transpose(1, 0, 2).reshape(E, E)).astype(bf)
    wo32 = np.ascontiguousarray(Wo).astype(bf)
    in_maps = []
    xT_b = [np.ascontiguousarray(xs[b].T).astype(bf) for b in range(B)]
    for c in range(NCORES):
        b, half = divmod(c, 2)
        xT = xT_b[b] if half == 0 else np.ascontiguousarray(
            np.concatenate([xT_b[b][:, SL:], xT_b[b][:, :SL]], axis=1))
        in_maps.append({"xT": xT, "wq": wq16, "wk": wk16, "wv": wv16, "wo": wo32})
    return in_maps


def kernel(xs, Wq, bq, Wk, bk, Wv, bv, Wo, bo):
    from concourse.bass_utils import run_bass_kernel_spmd

    if "nc" not in _cache:
        _cache["nc"] = _build()
    nc = _cache["nc"]

    xs = np.asarray(xs, dtype=np.float32)
    Wq = np.asarray(Wq, dtype=np.float32)
    Wk = np.asarray(Wk, dtype=np.float32)
    Wv = np.asarray(Wv, dtype=np.float32)
    Wo = np.asarray(Wo, dtype=np.float32)
    bq = np.asarray(bq, dtype=np.float32)
    bk = np.asarray(bk, dtype=np.float32)
    bv = np.asarray(bv, dtype=np.float32)
    bo = np.asarray(bo, dtype=np.float32)
    assert not (np.any(bq) or np.any(bk)), "nonzero bq/bk not supported"

    in_maps = _prep(xs, Wq, Wk, Wv, Wo)

    trace = bool(int(os.environ.get("BASS_KERNEL_TRACE", "0")))
    if trace:
        try:
            import antenv.axon_hooks  # noqa: F401  (registered by the harness)
        except ImportError:
            trace = False
    kw = dict(trace=True, trace_cores=[0]) if trace else {}
    res = run_bass_kernel_spmd(nc, in_maps, core_ids=list(range(NCORES)), **kw)
    if trace and res.exec_time_ns is not None:
        print(f"HW exec time: {res.exec_time_ns} ns")
        if res.instructions_and_trace is not None:
            print("trace:", res.instructions_and_trace[1])

    out = np.empty((B, S, E), dtype=np.float32)
    for c in range(NCORES):
        b, half = divmod(c, 2)
        out[b, half * SL:(half + 1) * SL, :] = res.results[c]["out"]

    # exact host-side correction for v/output biases (zero in this problem)
    if np.any(bv) or np.any(bo):
        out += bv.reshape(E) @ Wo + bo
    return out


# revision 22
# speedup vs baseline: 1.1257x; 1.0193x over previous
"""Multi-head attention Trainium2 kernel (8 NeuronCores).

Sharding: 8 cores = 4 batches x 2 sequence-halves (data parallel, no
collectives).  Each core computes, for its (batch, s-half):
  - q/k/v projections for all 16 heads in transposed layout
  - scoresT = k_h^T-chunks (x) q_h  per head, softmax without max-subtraction
    (scores are bounded ~ +-3.5 for this problem's input distribution)
  - ctxT via col-packed matmuls with v as the stationary operand; the softmax
    denominator comes from a ones-stationary matmul replicated across
    partitions so the division is partition-aligned on VectorE
  - row-wise output projection out = concat(ctx) @ Wo
Host side: transpose/cast/shard inputs, gather per-core outputs.

Bias handling: bq/bk/bv/bo are all zero for this problem.  bo and bv have
exact host-side corrections (softmax rows sum to 1 so a v-bias shifts ctx by
exactly bv); bq/bk would require a device change and are asserted zero.
"""

import os

import numpy as np
import ml_dtypes

B, S, E, H, DH = 4, 2048, 1024, 16, 64
SL = S // 2          # per-core local sequence (s-half)
NE = E // 128        # e-tiles (contraction)
NT = S // 128        # t-tiles (keys)
NP = H // 2          # head-pairs; pair j = heads (2j, 2j+1) = hd rows 128j..128j+128
NCORES = 8

_cache = {}


def _build(reps=1):
    import concourse.mybir as mybir
    import concourse.tile as tile
    from concourse import bacc
    from contextlib import ExitStack

    f32 = mybir.dt.float32
    f32r = mybir.dt.float32r
    bf16 = mybir.dt.bfloat16
    EXP = mybir.ActivationFunctionType.Exp

    nc = bacc.Bacc("TRN2", target_bir_lowering=False, debug=False,
                   num_devices=NCORES)

    # xT column order per core: own s-half first (t-permutation is harmless
    # for attention sums), so the q-projection always reads columns 0:SL.
    xT_d = nc.dram_tensor("xT", [E, S], bf16, kind="ExternalInput")
    wq_d = nc.dram_tensor("wq", [E, E], bf16, kind="ExternalInput")
    wk_d = nc.dram_tensor("wk", [E, E], bf16, kind="ExternalInput")
    wv_d = nc.dram_tensor("wv", [E, E], bf16, kind="ExternalInput")
    wo_d = nc.dram_tensor("wo", [E, E], bf16, kind="ExternalInput")
    out_d = nc.dram_tensor("out", [SL, E], f32, kind="ExternalOutput")

    with tile.TileContext(nc) as tc, ExitStack() as top:
        singles = top.enter_context(tc.tile_pool(name="singles", bufs=1))
        sb_out = top.enter_context(tc.tile_pool(name="sb_out", bufs=2))
        ps_s = top.enter_context(tc.tile_pool(name="ps_s", bufs=2, space="PSUM"))
        ps_qk = top.enter_context(tc.tile_pool(name="ps_qk", bufs=2, space="PSUM"))
        ps_cd = top.enter_context(tc.tile_pool(name="ps_cd", bufs=2, space="PSUM"))

        xT_sb = singles.tile([128, NE, S], bf16)
        qT_sb = singles.tile([128, NP, SL], bf16)
        ctx_sb = singles.tile([128, NP, SL], bf16)
        wo_sb = singles.tile([128, NP, E], bf16)
        xT_r = xT_d.rearrange("(eo ei) t -> ei eo t", ei=128)
        wo_r = wo_d.rearrange("(ho hi) e -> hi ho e", hi=128)

        def load_xT():
            for e in range(NE):
                for h2 in range(2):
                    eng = nc.sync if (2 * e + h2) % 2 == 0 else nc.gpsimd
                    eng.dma_start(out=xT_sb[:, e, h2 * 1024:(h2 + 1) * 1024],
                                  in_=xT_r[:, e, h2 * 1024:(h2 + 1) * 1024])

        for _rep in range(reps):
          with ExitStack() as attn:
            sb_w = attn.enter_context(tc.tile_pool(name="sb_w", bufs=2))
            sb_kt = attn.enter_context(tc.tile_pool(name="sb_kt", bufs=2))
            sb_v = attn.enter_context(tc.tile_pool(name="sb_v", bufs=2))
            sb_pt = attn.enter_context(tc.tile_pool(name="sb_pt", bufs=4))
            sb_nm = attn.enter_context(tc.tile_pool(name="sb_nm", bufs=1))

            v_tiles = {}
            kT_tiles = {}
            w0 = {}
            w0["wv"] = sb_w.tile([128, NE, 512], bf16, tag="wv", name="wv0")
            wv_r0 = wv_d[:, 0:512].rearrange("(eo ei) h -> ei eo h", ei=128)
            for e in range(NE):
                nc.gpsimd.dma_start(out=w0["wv"][:, e, :], in_=wv_r0[:, e, :])
            w0["wq"] = sb_w.tile([128, NE, 128], bf16, tag="wq", name="wq0")
            w0["wk"] = sb_w.tile([128, NE, 128], bf16, tag="wk", name="wk0")
            wq_r0 = wq_d[:, 0:128].rearrange("(eo ei) h -> ei eo h", ei=128)
            wk_r0 = wk_d[:, 0:128].rearrange("(eo ei) h -> ei eo h", ei=128)
            for e in range(NE):
                nc.gpsimd.dma_start(out=w0["wq"][:, e, :], in_=wq_r0[:, e, :])
                nc.gpsimd.dma_start(out=w0["wk"][:, e, :], in_=wk_r0[:, e, :])
            if _rep == 0:
                load_xT()

            def project_v(o):
                # project v for octet o (8 heads = pairs 4o..4o+3).
                # v_sb slot hh holds [v_h | ones] for even heads and
                # [ones | v_h] for odd heads: the ctx matmul then yields
                # ctx and the softmax denominator in one pass.
                if o == 0:
                    wv_sb = w0.pop("wv")
                else:
                    wv_sb = sb_w.tile([128, NE, 512], bf16, tag="wv")
                    wv_r = wv_d[:, o * 512:(o + 1) * 512].rearrange(
                        "(eo ei) h -> ei eo h", ei=128)
                    for e in range(NE):
                        nc.gpsimd.dma_start(out=wv_sb[:, e, :], in_=wv_r[:, e, :])
                v_sb = sb_v.tile([128, NT, 8, 128], bf16, tag="v")
                nc.vector.memset(v_sb[:, :, 0:8:2, 64:128], 1.0)
                nc.vector.memset(v_sb[:, :, 1:8:2, 0:64], 1.0)
                for t in range(NT):
                    ps_v = ps_s.tile([128, 512], f32, tag="ps_s")
                    for e in range(NE):
                        nc.tensor.matmul(
                            ps_v[:], xT_sb[:, e, t * 128:(t + 1) * 128],
                            wv_sb[:, e, :],
                            start=(e == 0), stop=(e == NE - 1))
                    pv = ps_v.rearrange("p (h d) -> p h d", d=64)
                    nc.vector.tensor_copy(v_sb[:, t, 0:8:2, 0:64], pv[:, 0:8:2, :])
                    nc.vector.tensor_copy(v_sb[:, t, 1:8:2, 64:128], pv[:, 1:8:2, :])
                v_tiles[o] = v_sb

            def project_qk(j):
                if j % 4 == 0:
                    project_v(j // 4)
                if j == 0:
                    wq_sb = w0.pop("wq")
                    wk_sb = w0.pop("wk")
                else:
                    wq_sb = sb_w.tile([128, NE, 128], bf16, tag="wq")
                    wk_sb = sb_w.tile([128, NE, 128], bf16, tag="wk")
                    wq_r = wq_d[:, j * 128:(j + 1) * 128].rearrange(
                        "(eo ei) h -> ei eo h", ei=128)
                    wk_r = wk_d[:, j * 128:(j + 1) * 128].rearrange(
                        "(eo ei) h -> ei eo h", ei=128)
                    for e in range(NE):
                        nc.gpsimd.dma_start(out=wq_sb[:, e, :], in_=wq_r[:, e, :])
                        nc.gpsimd.dma_start(out=wk_sb[:, e, :], in_=wk_r[:, e, :])
                # spread the wo load: one pair-row chunk per pair iteration
                nc.gpsimd.dma_start(out=wo_sb[:, j, :], in_=wo_r[:, j, :])

                # qT[j] : [128 hd, SL]
                for sc in range(2):
                    ps_q = ps_qk.tile([128, 512], f32, tag="ps_qk")
                    for e in range(NE):
                        nc.tensor.matmul(
                            ps_q[:], wq_sb[:, e, :],
                            xT_sb[:, e, sc * 512:(sc + 1) * 512],
                            start=(e == 0), stop=(e == NE - 1))
                    nc.vector.tensor_copy(qT_sb[:, j, sc * 512:(sc + 1) * 512], ps_q[:])

                # kT[j] : [128 hd, S]
                kT_sb = sb_kt.tile([128, S], bf16, tag="kt")
                for tch in range(4):
                    ps_k = ps_qk.tile([128, 512], f32, tag="ps_qk")
                    for e in range(NE):
                        nc.tensor.matmul(
                            ps_k[:], wk_sb[:, e, :],
                            xT_sb[:, e, tch * 512:(tch + 1) * 512],
                            start=(e == 0), stop=(e == NE - 1))
                    nc.vector.tensor_copy(kT_sb[:, tch * 512:(tch + 1) * 512], ps_k[:])
                kT_tiles[j] = kT_sb

            project_qk(0)
            for j in range(NP):
                if j + 1 < NP:
                    project_qk(j + 1)
                jj = j % 4
                v_sb = v_tiles[j // 4]
                kT_sb = kT_tiles.pop(j)

                # attention for pair j, per s-chunk of 512
                for sc in range(2):
                    # ps_ca: head A = [v|ones] -> ctx rows 0:64, denom rows 64:128
                    # ps_cb: head B = [ones|v] -> denom rows 0:64, ctx rows 64:128
                    ps_ca = ps_cd.tile([128, 512], f32, tag="ps_cd")
                    ps_cb = ps_cd.tile([128, 512], f32, tag="ps_cd")
                    for t in range(NT):
                        ps_sc = ps_s.tile([128, 1024], f32, tag="ps_s")
                        # scoresT for heads A/B, row-packed (K=64 each)
                        nc.tensor.matmul(
                            ps_sc[:, 0:512],
                            kT_sb[0:64, t * 128:(t + 1) * 128],
                            qT_sb[0:64, j, sc * 512:(sc + 1) * 512],
                            start=True, stop=True, tile_position=(0, 0))
                        nc.tensor.matmul(
                            ps_sc[:, 512:1024],
                            kT_sb[64:128, t * 128:(t + 1) * 128],
                            qT_sb[64:128, j, sc * 512:(sc + 1) * 512],
                            start=True, stop=True, tile_position=(64, 0))
                        pt_t = sb_pt.tile([128, 1024], bf16, tag="pt")
                        nc.scalar.activation(pt_t[:], ps_sc[:], EXP, scale=0.125)
                        stt, stp = (t == 0), (t == NT - 1)
                        nc.tensor.matmul(
                            ps_ca[:], v_sb[:, t, jj * 2, :], pt_t[:, 0:512],
                            start=stt, stop=stp)
                        nc.tensor.matmul(
                            ps_cb[:], v_sb[:, t, jj * 2 + 1, :], pt_t[:, 512:1024],
                            start=stt, stop=stp)
                    # evacuate both psums in one copy each (releases the
                    # ps_cd slots for the next s-chunk immediately), then
                    # normalize off the SBUF copies
                    tA = sb_nm.tile([128, 512], f32, tag="tA")
                    tB = sb_nm.tile([128, 512], f32, tag="tB")
                    nc.vector.tensor_copy(tA[:, :], ps_ca[:, :])
                    nc.vector.tensor_copy(tB[:, :], ps_cb[:, :])
                    # head A: denom replicated at rows 64:128; move one row to
                    # partition 0, reciprocal, broadcast to rows 0:64
                    rA = sb_nm.tile([1, 512], f32, tag="rA")
                    rbA = sb_nm.tile([64, 512], f32, tag="rbA")
                    nc.sync.dma_start(out=rA[0:1, :], in_=tA[64:65, :])
                    nc.vector.reciprocal_approx_fast(rA[0:1, :], rA[0:1, :])
                    nc.gpsimd.partition_broadcast(rbA[:, :], rA[0:1, :])
                    nc.vector.tensor_mul(
                        ctx_sb[0:64, j, sc * 512:(sc + 1) * 512],
                        tA[0:64, :], rbA[:, :])
                    # head B: denom at row 0 already
                    rB = sb_nm.tile([1, 512], f32, tag="rB")
                    rbB = sb_nm.tile([128, 512], f32, tag="rbB")
                    nc.vector.reciprocal_approx_fast(rB[0:1, :], tB[0:1, :])
                    nc.gpsimd.partition_broadcast(rbB[:, :], rB[0:1, :])
                    nc.vector.tensor_mul(
                        ctx_sb[64:128, j, sc * 512:(sc + 1) * 512],
                        tB[64:128, :], rbB[64:128, :])

          # output projection: out[s, e] = sum_j ctxT_j^T @ wo_j
          if True:
            for stile in range(8):
                for sc in range(2):
                    ps_o = ps_qk.tile([128, 512], f32, tag="ps_qk")
                    for j in range(NP):
                        nc.tensor.matmul(
                            ps_o[:], ctx_sb[:, j, stile * 128:(stile + 1) * 128],
                            wo_sb[:, j, sc * 512:(sc + 1) * 512],
                            start=(j == 0), stop=(j == NP - 1))
                    ot = sb_out.tile([128, 512], f32, tag="out")
                    nc.vector.tensor_copy(ot[:], ps_o[:])
                    nc.sync.dma_start(
                        out=out_d[stile * 128:(stile + 1) * 128,
                                  sc * 512:(sc + 1) * 512],
                        in_=ot[:])

    nc.compile()
    return nc


def _prep(xs, Wq, Wk, Wv, Wo):
    bf = ml_dtypes.bfloat16
    wq16 = np.ascontiguousarray(Wq.transpose(1, 0, 2).reshape(E, E)).astype(bf)
    wk16 = np.ascontiguousarray(Wk.transpose(1, 0, 2).reshape(E, E)).astype(bf)
    wv16 = np.ascontiguousarray(Wv.transpose(1, 0, 2).reshape(E, E)).astype(bf)
    wo32 = np.ascontiguousarray(Wo).astype(bf)
    in_maps = []
    xT_b = [np.ascontiguousarray(xs[b].T).astype(bf) for b in range(B)]
    for c in range(NCORES):
        b, half = divmod(c, 2)
        xT = xT_b[b] if half == 0 else np.ascontiguousarray(
            np.concatenate([xT_b[b][:, SL:], xT_b[b][:, :SL]], axis=1))
        in_maps.append({"xT": xT, "wq": wq16, "wk": wk16, "wv": wv16, "wo": wo32})
    return in_maps


def kernel(xs, Wq, bq, Wk, bk, Wv, bv, Wo, bo):
    from concourse.bass_utils import run_bass_kernel_spmd

    if "nc" not in _cache:
        _cache["nc"] = _build()
    nc = _cache["nc"]

    xs = np.asarray(xs, dtype=np.float32)
    Wq = np.asarray(Wq, dtype=np.float32)
    Wk = np.asarray(Wk, dtype=np.float32)
    Wv = np.asarray(Wv, dtype=np.float32)
    Wo = np.asarray(Wo, dtype=np.float32)
    bq = np.asarray(bq, dtype=np.float32)
    bk = np.asarray(bk, dtype=np.float32)
    bv = np.asarray(bv, dtype=np.float32)
    bo = np.asarray(bo, dtype=np.float32)
    assert not (np.any(bq) or np.any(bk)), "nonzero bq/bk not supported"

    in_maps = _prep(xs, Wq, Wk, Wv, Wo)

    trace = bool(int(os.environ.get("BASS_KERNEL_TRACE", "0")))
    if trace:
        try:
            import antenv.axon_hooks  # noqa: F401  (registered by the harness)
        except ImportError:
            trace = False
    kw = dict(trace=True, trace_cores=[0]) if trace else {}
    res = run_bass_kernel_spmd(nc, in_maps, core_ids=list(range(NCORES)), **kw)
    if trace and res.exec_time_ns is not None:
        print(f"HW exec time: {res.exec_time_ns} ns")
        if res.instructions_and_trace is not None:
            print("trace:", res.instructions_and_trace[1])

    out = np.empty((B, S, E), dtype=np.float32)
    for c in range(NCORES):
        b, half = divmod(c, 2)
        out[b, half * SL:(half + 1) * SL, :] = res.results[c]["out"]

    # exact host-side correction for v/output biases (zero in this problem)
    if np.any(bv) or np.any(bo):
        out += bv.reshape(E) @ Wo + bo
    return out


# revision 23
# speedup vs baseline: 1.1377x; 1.0106x over previous
"""Multi-head attention Trainium2 kernel (8 NeuronCores).

Sharding: 8 cores = 4 batches x 2 sequence-halves (data parallel, no
collectives).  Each core computes, for its (batch, s-half):
  - q/k/v projections for all 16 heads in transposed layout
  - scoresT = k_h^T-chunks (x) q_h  per head, softmax without max-subtraction
    (scores are bounded ~ +-3.5 for this problem's input distribution)
  - ctxT via col-packed matmuls with v as the stationary operand; the softmax
    denominator comes from a ones-stationary matmul replicated across
    partitions so the division is partition-aligned on VectorE
  - row-wise output projection out = concat(ctx) @ Wo
Host side: transpose/cast/shard inputs, gather per-core outputs.

Bias handling: bq/bk/bv/bo are all zero for this problem.  bo and bv have
exact host-side corrections (softmax rows sum to 1 so a v-bias shifts ctx by
exactly bv); bq/bk would require a device change and are asserted zero.
"""

import os

import numpy as np
import ml_dtypes

B, S, E, H, DH = 4, 2048, 1024, 16, 64
SL = S // 2          # per-core local sequence (s-half)
NE = E // 128        # e-tiles (contraction)
NT = S // 128        # t-tiles (keys)
NP = H // 2          # head-pairs; pair j = heads (2j, 2j+1) = hd rows 128j..128j+128
NCORES = 8

_cache = {}


def _build(reps=1):
    import concourse.mybir as mybir
    import concourse.tile as tile
    from concourse import bacc
    from contextlib import ExitStack

    f32 = mybir.dt.float32
    f32r = mybir.dt.float32r
    bf16 = mybir.dt.bfloat16
    EXP = mybir.ActivationFunctionType.Exp

    nc = bacc.Bacc("TRN2", target_bir_lowering=False, debug=False,
                   num_devices=NCORES)

    # xT column order per core: own s-half first (t-permutation is harmless
    # for attention sums), so the q-projection always reads columns 0:SL.
    xT_d = nc.dram_tensor("xT", [E, S], bf16, kind="ExternalInput")
    wq_d = nc.dram_tensor("wq", [E, E], bf16, kind="ExternalInput")
    wk_d = nc.dram_tensor("wk", [E, E], bf16, kind="ExternalInput")
    wv_d = nc.dram_tensor("wv", [E, E], bf16, kind="ExternalInput")
    wo_d = nc.dram_tensor("wo", [E, E], bf16, kind="ExternalInput")
    out_d = nc.dram_tensor("out", [SL, E], f32, kind="ExternalOutput")

    with tile.TileContext(nc) as tc, ExitStack() as top:
        singles = top.enter_context(tc.tile_pool(name="singles", bufs=1))
        sb_out = top.enter_context(tc.tile_pool(name="sb_out", bufs=2))
        ps_s = top.enter_context(tc.tile_pool(name="ps_s", bufs=2, space="PSUM"))
        ps_qk = top.enter_context(tc.tile_pool(name="ps_qk", bufs=2, space="PSUM"))
        ps_cd = top.enter_context(tc.tile_pool(name="ps_cd", bufs=2, space="PSUM"))

        xT_sb = singles.tile([128, NE, S], bf16)
        qT_sb = singles.tile([128, NP, SL], bf16)
        ctx_sb = singles.tile([128, NP, SL], bf16)
        wo_sb = singles.tile([128, NP, E], bf16)
        xT_r = xT_d.rearrange("(eo ei) t -> ei eo t", ei=128)
        wo_r = wo_d.rearrange("(ho hi) e -> hi ho e", hi=128)

        def load_xT():
            for e in range(NE):
                for h2 in range(2):
                    eng = nc.sync if (2 * e + h2) % 2 == 0 else nc.gpsimd
                    eng.dma_start(out=xT_sb[:, e, h2 * 1024:(h2 + 1) * 1024],
                                  in_=xT_r[:, e, h2 * 1024:(h2 + 1) * 1024])

        for _rep in range(reps):
          with ExitStack() as attn:
            sb_w = attn.enter_context(tc.tile_pool(name="sb_w", bufs=2))
            sb_kt = attn.enter_context(tc.tile_pool(name="sb_kt", bufs=3))
            sb_v = attn.enter_context(tc.tile_pool(name="sb_v", bufs=2))
            sb_pt = attn.enter_context(tc.tile_pool(name="sb_pt", bufs=5))
            sb_nm = attn.enter_context(tc.tile_pool(name="sb_nm", bufs=1))

            v_tiles = {}
            kT_tiles = {}
            w0 = {}
            w0["wv"] = sb_w.tile([128, NE, 512], bf16, tag="wv", name="wv0")
            wv_r0 = wv_d[:, 0:512].rearrange("(eo ei) h -> ei eo h", ei=128)
            for e in range(NE):
                nc.gpsimd.dma_start(out=w0["wv"][:, e, :], in_=wv_r0[:, e, :])
            w0["wq"] = sb_w.tile([128, NE, 128], bf16, tag="wq", name="wq0")
            w0["wk"] = sb_w.tile([128, NE, 128], bf16, tag="wk", name="wk0")
            wq_r0 = wq_d[:, 0:128].rearrange("(eo ei) h -> ei eo h", ei=128)
            wk_r0 = wk_d[:, 0:128].rearrange("(eo ei) h -> ei eo h", ei=128)
            for e in range(NE):
                nc.gpsimd.dma_start(out=w0["wq"][:, e, :], in_=wq_r0[:, e, :])
                nc.gpsimd.dma_start(out=w0["wk"][:, e, :], in_=wk_r0[:, e, :])
            if _rep == 0:
                load_xT()

            def project_v(o):
                # project v for octet o (8 heads = pairs 4o..4o+3).
                # v_sb slot hh holds [v_h | ones] for even heads and
                # [ones | v_h] for odd heads: the ctx matmul then yields
                # ctx and the softmax denominator in one pass.
                if o == 0:
                    wv_sb = w0.pop("wv")
                else:
                    wv_sb = sb_w.tile([128, NE, 512], bf16, tag="wv")
                    wv_r = wv_d[:, o * 512:(o + 1) * 512].rearrange(
                        "(eo ei) h -> ei eo h", ei=128)
                    for e in range(NE):
                        nc.gpsimd.dma_start(out=wv_sb[:, e, :], in_=wv_r[:, e, :])
                v_sb = sb_v.tile([128, NT, 8, 128], bf16, tag="v")
                nc.vector.memset(v_sb[:, :, 0:8:2, 64:128], 1.0)
                nc.vector.memset(v_sb[:, :, 1:8:2, 0:64], 1.0)
                for t in range(NT):
                    ps_v = ps_s.tile([128, 512], f32, tag="ps_s")
                    for e in range(NE):
                        nc.tensor.matmul(
                            ps_v[:], xT_sb[:, e, t * 128:(t + 1) * 128],
                            wv_sb[:, e, :],
                            start=(e == 0), stop=(e == NE - 1))
                    pv = ps_v.rearrange("p (h d) -> p h d", d=64)
                    nc.vector.tensor_copy(v_sb[:, t, 0:8:2, 0:64], pv[:, 0:8:2, :])
                    nc.vector.tensor_copy(v_sb[:, t, 1:8:2, 64:128], pv[:, 1:8:2, :])
                v_tiles[o] = v_sb

            def project_qk(j):
                if j % 4 == 0:
                    project_v(j // 4)
                if j == 0:
                    wq_sb = w0.pop("wq")
                    wk_sb = w0.pop("wk")
                else:
                    wq_sb = sb_w.tile([128, NE, 128], bf16, tag="wq")
                    wk_sb = sb_w.tile([128, NE, 128], bf16, tag="wk")
                    wq_r = wq_d[:, j * 128:(j + 1) * 128].rearrange(
                        "(eo ei) h -> ei eo h", ei=128)
                    wk_r = wk_d[:, j * 128:(j + 1) * 128].rearrange(
                        "(eo ei) h -> ei eo h", ei=128)
                    for e in range(NE):
                        nc.gpsimd.dma_start(out=wq_sb[:, e, :], in_=wq_r[:, e, :])
                        nc.gpsimd.dma_start(out=wk_sb[:, e, :], in_=wk_r[:, e, :])
                # spread the wo load: one pair-row chunk per pair iteration
                nc.gpsimd.dma_start(out=wo_sb[:, j, :], in_=wo_r[:, j, :])

                # qT[j] : [128 hd, SL]
                for sc in range(2):
                    ps_q = ps_qk.tile([128, 512], f32, tag="ps_qk")
                    for e in range(NE):
                        nc.tensor.matmul(
                            ps_q[:], wq_sb[:, e, :],
                            xT_sb[:, e, sc * 512:(sc + 1) * 512],
                            start=(e == 0), stop=(e == NE - 1))
                    nc.vector.tensor_copy(qT_sb[:, j, sc * 512:(sc + 1) * 512], ps_q[:])

                # kT[j] : [128 hd, S]
                kT_sb = sb_kt.tile([128, S], bf16, tag="kt")
                for tch in range(4):
                    ps_k = ps_qk.tile([128, 512], f32, tag="ps_qk")
                    for e in range(NE):
                        nc.tensor.matmul(
                            ps_k[:], wk_sb[:, e, :],
                            xT_sb[:, e, tch * 512:(tch + 1) * 512],
                            start=(e == 0), stop=(e == NE - 1))
                    nc.vector.tensor_copy(kT_sb[:, tch * 512:(tch + 1) * 512], ps_k[:])
                kT_tiles[j] = kT_sb

            project_qk(0)
            for j in range(NP):
                if j + 1 < NP:
                    project_qk(j + 1)
                jj = j % 4
                v_sb = v_tiles[j // 4]
                kT_sb = kT_tiles.pop(j)

                # attention for pair j, per s-chunk of 512
                for sc in range(2):
                    # ps_ca: head A = [v|ones] -> ctx rows 0:64, denom rows 64:128
                    # ps_cb: head B = [ones|v] -> denom rows 0:64, ctx rows 64:128
                    ps_ca = ps_cd.tile([128, 512], f32, tag="ps_cd")
                    ps_cb = ps_cd.tile([128, 512], f32, tag="ps_cd")
                    for t in range(NT):
                        ps_sc = ps_s.tile([128, 1024], f32, tag="ps_s")
                        # scoresT for heads A/B, row-packed (K=64 each)
                        nc.tensor.matmul(
                            ps_sc[:, 0:512],
                            kT_sb[0:64, t * 128:(t + 1) * 128],
                            qT_sb[0:64, j, sc * 512:(sc + 1) * 512],
                            start=True, stop=True, tile_position=(0, 0))
                        nc.tensor.matmul(
                            ps_sc[:, 512:1024],
                            kT_sb[64:128, t * 128:(t + 1) * 128],
                            qT_sb[64:128, j, sc * 512:(sc + 1) * 512],
                            start=True, stop=True, tile_position=(64, 0))
                        pt_t = sb_pt.tile([128, 1024], bf16, tag="pt")
                        nc.scalar.activation(pt_t[:], ps_sc[:], EXP, scale=0.125)
                        stt, stp = (t == 0), (t == NT - 1)
                        nc.tensor.matmul(
                            ps_ca[:], v_sb[:, t, jj * 2, :], pt_t[:, 0:512],
                            start=stt, stop=stp)
                        nc.tensor.matmul(
                            ps_cb[:], v_sb[:, t, jj * 2 + 1, :], pt_t[:, 512:1024],
                            start=stt, stop=stp)
                    # evacuate both psums in one copy each (releases the
                    # ps_cd slots for the next s-chunk immediately), then
                    # normalize off the SBUF copies
                    tA = sb_nm.tile([128, 512], f32, tag="tA")
                    tB = sb_nm.tile([128, 512], f32, tag="tB")
                    nc.vector.tensor_copy(tA[:, :], ps_ca[:, :])
                    nc.vector.tensor_copy(tB[:, :], ps_cb[:, :])
                    # head A: denom replicated at rows 64:128; move one row to
                    # partition 0, reciprocal, broadcast to rows 0:64
                    rA = sb_nm.tile([1, 512], f32, tag="rA")
                    rbA = sb_nm.tile([64, 512], f32, tag="rbA")
                    nc.sync.dma_start(out=rA[0:1, :], in_=tA[64:65, :])
                    nc.vector.reciprocal_approx_fast(rA[0:1, :], rA[0:1, :])
                    nc.gpsimd.partition_broadcast(rbA[:, :], rA[0:1, :])
                    nc.vector.tensor_mul(
                        ctx_sb[0:64, j, sc * 512:(sc + 1) * 512],
                        tA[0:64, :], rbA[:, :])
                    # head B: denom at row 0 already
                    rB = sb_nm.tile([1, 512], f32, tag="rB")
                    rbB = sb_nm.tile([128, 512], f32, tag="rbB")
                    nc.vector.reciprocal_approx_fast(rB[0:1, :], tB[0:1, :])
                    nc.gpsimd.partition_broadcast(rbB[:, :], rB[0:1, :])
                    nc.vector.tensor_mul(
                        ctx_sb[64:128, j, sc * 512:(sc + 1) * 512],
                        tB[64:128, :], rbB[64:128, :])

          # output projection: out[s, e] = sum_j ctxT_j^T @ wo_j
          if True:
            for stile in range(8):
                for sc in range(2):
                    ps_o = ps_qk.tile([128, 512], f32, tag="ps_qk")
                    for j in range(NP):
                        nc.tensor.matmul(
                            ps_o[:], ctx_sb[:, j, stile * 128:(stile + 1) * 128],
                            wo_sb[:, j, sc * 512:(sc + 1) * 512],
                            start=(j == 0), stop=(j == NP - 1))
                    ot = sb_out.tile([128, 512], f32, tag="out")
                    nc.vector.tensor_copy(ot[:], ps_o[:])
                    nc.sync.dma_start(
                        out=out_d[stile * 128:(stile + 1) * 128,
                                  sc * 512:(sc + 1) * 512],
                        in_=ot[:])

    nc.compile()
    return nc


def _prep(xs, Wq, Wk, Wv, Wo):
    bf = ml_dtypes.bfloat16
    wq16 = np.ascontiguousarray(Wq.transpose(1, 0, 2).reshape(E, E)).astype(bf)
    wk16 = np.ascontiguousarray(Wk.transpose(1, 0, 2).reshape(E, E)).astype(bf)
    wv16 = np.ascontiguousarray(Wv.transpose(1, 0, 2).reshape(E, E)).astype(bf)
    wo32 = np.ascontiguousarray(Wo).astype(bf)
    in_maps = []
    xT_b = [np.ascontiguousarray(xs[b].T).astype(bf) for b in range(B)]
    for c in range(NCORES):
        b, half = divmod(c, 2)
        xT = xT_b[b] if half == 0 else np.ascontiguousarray(
            np.concatenate([xT_b[b][:, SL:], xT_b[b][:, :SL]], axis=1))
        in_maps.append({"xT": xT, "wq": wq16, "wk": wk16, "wv": wv16, "wo": wo32})
    return in_maps


def kernel(xs, Wq, bq, Wk, bk, Wv, bv, Wo, bo):
    from concourse.bass_utils import run_bass_kernel_spmd

    if "nc" not in _cache:
        _cache["nc"] = _build()
    nc = _cache["nc"]

    xs = np.asarray(xs, dtype=np.float32)
    Wq = np.asarray(Wq, dtype=np.float32)
    Wk = np.asarray(Wk, dtype=np.float32)
    Wv = np.asarray(Wv, dtype=np.float32)
    Wo = np.asarray(Wo, dtype=np.float32)
    bq = np.asarray(bq, dtype=np.float32)
    bk = np.asarray(bk, dtype=np.float32)
    bv = np.asarray(bv, dtype=np.float32)
    bo = np.asarray(bo, dtype=np.float32)
    assert not (np.any(bq) or np.any(bk)), "nonzero bq/bk not supported"

    in_maps = _prep(xs, Wq, Wk, Wv, Wo)

    trace = bool(int(os.environ.get("BASS_KERNEL_TRACE", "0")))
    if trace:
        try:
            import antenv.axon_hooks  # noqa: F401  (registered by the harness)
        except ImportError:
            trace = False
    kw = dict(trace=True, trace_cores=[0]) if trace else {}
    res = run_bass_kernel_spmd(nc, in_maps, core_ids=list(range(NCORES)), **kw)
    if trace and res.exec_time_ns is not None:
        print(f"HW exec time: {res.exec_time_ns} ns")
        if res.instructions_and_trace is not None:
            print("trace:", res.instructions_and_trace[1])

    out = np.empty((B, S, E), dtype=np.float32)
    for c in range(NCORES):
        b, half = divmod(c, 2)
        out[b, half * SL:(half + 1) * SL, :] = res.results[c]["out"]

    # exact host-side correction for v/output biases (zero in this problem)
    if np.any(bv) or np.any(bo):
        out += bv.reshape(E) @ Wo + bo
    return out
